# revision 51
# baseline (speedup 1.0000x reference)
"""Trainium2 Bass kernel for nn_CustomDetector (FPN + RetinaNet-style heads).

Data-parallel over the batch: 8 images -> 8 NeuronCores, one image per core.
Each core runs the full FPN (1x1 laterals + top-down nearest-2x fusion + 3x3
convs) and the shared cls/reg towers on its image, producing the final
[5376, 252] (cls|reg, pixel-major) tensor directly on device via PE
transposes.  Convs are expressed as 9 shifted bf16 matmuls accumulating in
PSUM (moving operand = strided activation windows from a zero-padded SBUF
buffer); weights are pre-packed on host into [K, M]-stationary layout.
Anchors are a pure host-side constant.
"""
import sys

import numpy as np

try:
    import concourse.bass as bass
except ImportError:  # fresh grading dir: point at the repo the env ships
    for p in ("/opt/trn_rl_repo", "/root/.axon_site/_ro/trn_rl_repo"):
        if p not in sys.path:
            sys.path.append(p)
    import concourse.bass as bass

import ml_dtypes

import concourse.mybir as mybir
import concourse.tile as tile
from concourse import bacc
from concourse.bass_utils import run_bass_kernel_spmd
from concourse.masks import make_identity

BF16 = mybir.dt.bfloat16
F32 = mybir.dt.float32
AluOp = mybir.AluOpType

N_CORES = 8
A, NC_CLS = 3, 80
C = 256  # FPN channel width
MO = C // 128  # output-channel chunks for 256-wide convs
# (H, W, lateral input channels, output row offset)
LEVELS = [(64, 64, 512, 0), (32, 32, 1024, 4096), (16, 16, 2048, 5120)]
TOTAL_PX = 5376
OUT_CH = A * NC_CLS + A * 4  # 240 + 12 = 252
BIAS_NAMES = ("lb0", "lb1", "lb2", "fb0", "fb1", "fb2", "cb0", "cb1", "cb2",
              "rb0", "rb1")

# std-conv N-tile geometry per level: list of (r0, R) row groups with R*W<=512
def _row_groups(H, W, max_n=512):
    R = max(1, min(H, max_n // W))
    groups = []
    r0 = 0
    while r0 < H:
        r = min(R, H - r0)
        groups.append((r0, r))
        r0 += r
    return groups


def _build_nc():
    nc = bacc.Bacc(
        "TRN2", target_bir_lowering=False, debug=False, num_devices=N_CORES
    )

    # ---- DRAM parameters (per core) ----
    dram = {}
    for i, (H, W, CIN, _) in enumerate(LEVELS):
        dram[f"feat{i}"] = nc.dram_tensor(
            f"feat{i}", [128, CIN // 128, H * W], BF16, kind="ExternalInput"
        )
        dram[f"lw{i}"] = nc.dram_tensor(
            f"lw{i}", [128, CIN // 128, MO, 128], BF16, kind="ExternalInput"
        )
        dram[f"fw{i}"] = nc.dram_tensor(
            f"fw{i}", [128, MO, 9, MO, 128], BF16, kind="ExternalInput"
        )
    for name in ("cw0", "cw1", "rw0", "rw1"):
        dram[name] = nc.dram_tensor(
            name, [128, MO, 9, MO, 128], BF16, kind="ExternalInput"
        )
    dram["cw2"] = nc.dram_tensor(
        "cw2", [128, MO, 9, MO, 128], BF16, kind="ExternalInput"
    )  # 240 outs padded to 256
    dram["rw2"] = nc.dram_tensor(
        "rw2", [128, MO, 3, 128], BF16, kind="ExternalInput"
    )  # tap-packed, 32-aligned: pack j holds taps 4j+b at partition 32b
    # all biases merged into one tensor: 11 x [128, MO] planes + rb2 in
    # column 22 (rows 0..12)
    dram["biases"] = nc.dram_tensor(
        "biases", [128, 2 * len(BIAS_NAMES) + 1], F32, kind="ExternalInput"
    )
    out = nc.dram_tensor("out", [TOTAL_PX, OUT_CH], F32, kind="ExternalOutput")

    with tile.TileContext(nc) as tc:
        with (
            tc.tile_pool(name="wp", bufs=1) as wp,
            tc.tile_pool(name="lp", bufs=1) as lp,
            tc.tile_pool(name="ps", bufs=4, space="PSUM") as ps,
            tc.tile_pool(name="psr", bufs=3, space="PSUM") as psr,
            tc.tile_pool(name="pst", bufs=1, space="PSUM") as pst,
        ):
            # ---- constants + lateral weights first (consumption order) ----
            bias_t = wp.tile([128, 2 * len(BIAS_NAMES) + 1], F32,
                             name="bias_t")
            nc.sync.dma_start(bias_t[:], dram["biases"][:])
            bt = {
                name: bias_t[:, 2 * idx : 2 * idx + 2]
                for idx, name in enumerate(BIAS_NAMES)
            }
            rb2t = bias_t[0:12, 2 * len(BIAS_NAMES) : 2 * len(BIAS_NAMES) + 1]
            ident = wp.tile([128, 128], F32, name="ident")
            make_identity(nc, ident[:])
            wt = {}

            # ---- padded level buffers ----
            def zero_border(t, H, W):
                # on GpSimd: keeps DVE free for evacs/adds
                nc.gpsimd.memset(t[:, :, 0, :], 0.0)
                nc.gpsimd.memset(t[:, :, H + 1, :], 0.0)
                nc.gpsimd.memset(t[:, :, 1 : H + 1, 0], 0.0)
                nc.gpsimd.memset(t[:, :, 1 : H + 1, W + 1], 0.0)

            lpad = []
            for i, (H, W, _, _) in enumerate(LEVELS):
                t = lp.tile([128, MO, H + 2, W + 2], BF16, name=f"l{i}pad")
                zero_border(t, H, W)
                lpad.append(t)

            def conv3x3(src_pad, w, b, dst_pad, H, W, relu, name,
                        evac_dve=False):
                """src_pad [128, MO, H+2, W+2] -> dst_pad interior, 256->256."""
                for mo in range(MO):
                    for gi, (r0, R) in enumerate(_row_groups(H, W)):
                        acc = ps.tile([128, 512], F32, tag="ps",
                                      name=f"{name}_ps")
                        accv = acc[:, : R * W].rearrange(
                            "p (r w) -> p r w", r=R
                        )
                        i = 0
                        for ki in range(MO):
                            for t in range(9):
                                dy, dx = t // 3, t % 3
                                nc.tensor.matmul(
                                    acc[:, : R * W],
                                    w[:, ki, t, mo, :],
                                    src_pad[:, ki, r0 + dy : r0 + dy + R,
                                            dx : dx + W],
                                    start=(i == 0),
                                    stop=(i == MO * 9 - 1),
                                )
                                i += 1
                        dst = dst_pad[:, mo, 1 + r0 : 1 + r0 + R, 1 : 1 + W]
                        # split evacs between ACT and DVE; DVE only gets
                        # phases that don't collide with the reg tap-adds
                        if evac_dve:
                            if relu:
                                nc.vector.tensor_scalar(
                                    dst, accv, b[:, mo : mo + 1], 0.0,
                                    AluOp.add, AluOp.max,
                                )
                            else:
                                nc.vector.tensor_scalar_add(
                                    dst, accv, b[:, mo : mo + 1]
                                )
                        else:
                            func = (mybir.ActivationFunctionType.Relu if relu
                                    else mybir.ActivationFunctionType.Identity)
                            nc.scalar.activation(
                                dst, accv, func, bias=b[:, mo : mo + 1]
                            )

            # ---- laterals (1x1 convs) + top-down fusion ----
            feats = [None, None, None]

            def load_level(i, fpool):
                H, W, CIN, _ = LEVELS[i]
                KI = CIN // 128
                w = fpool.tile([128, KI, MO, 128], BF16, name=f"lw{i}t")
                wt[f"lw{i}"] = w
                f = fpool.tile([128, KI, H * W], BF16, name=f"feat{i}t")
                # ~512KB per DMA: amortize per-transfer overhead while
                # still letting the first matmuls start early
                kc_w = max(1, (1 << 19) // (128 * MO * 128 * 2))
                kc_f = max(1, (1 << 19) // (128 * H * W * 2))
                kc0 = min(kc_w, kc_f)
                for k0 in range(0, KI, kc0):
                    kc = min(kc0, KI - k0)
                    nc.sync.dma_start(
                        w[:, k0 : k0 + kc], dram[f"lw{i}"][:, k0 : k0 + kc]
                    )
                    nc.sync.dma_start(
                        f[:, k0 : k0 + kc], dram[f"feat{i}"][:, k0 : k0 + kc]
                    )
                feats[i] = f

            def load_weight(name):
                w = wp.tile([128, MO, 9, MO, 128], BF16, name=f"{name}t")
                nc.sync.dma_start(w[:], dram[name][:])
                wt[name] = w

            def lateral_level(i):
                # 1x1 lateral; the up2(coarser) fusion is added straight
                # into each PSUM group (fp32, group-local deps) before evac.
                H, W, CIN, _ = LEVELS[i]
                KI = CIN // 128
                for mo in range(MO):
                    for gi, (r0, R) in enumerate(_row_groups(H, W)):
                        n = R * W
                        px0 = r0 * W
                        acc = ps.tile([128, 512], F32, tag="ps",
                                      name=f"lat{i}_ps")
                        for ki in range(KI):
                            nc.tensor.matmul(
                                acc[:, :n],
                                wt[f"lw{i}"][:, ki, mo, :],
                                feats[i][:, ki, px0 : px0 + n],
                                start=(ki == 0),
                                stop=(ki == KI - 1),
                            )
                        accv = acc[:, :n].rearrange("p (r w) -> p r w", r=R)
                        if i < 2:
                            csrc = lpad[i + 1][
                                :, mo,
                                1 + r0 // 2 : 1 + (r0 + R) // 2,
                                1 : 1 + W // 2,
                            ]
                            for dy in range(2):
                                for dx in range(2):
                                    dst = accv[:, dy::2, dx::2]
                                    nc.vector.tensor_add(dst, dst, csrc)
                        nc.scalar.activation(
                            lpad[i][:, mo, 1 + r0 : 1 + r0 + R, 1 : 1 + W],
                            accv,
                            mybir.ActivationFunctionType.Identity,
                            bias=bt[f"lb{i}"][:, mo : mo + 1],
                        )

            with tc.tile_pool(name="fpA", bufs=1) as fpA:
                load_level(2, fpA)
                load_level(1, fpA)
                lateral_level(2)
                lateral_level(1)

            # ---- per-level FPN 3x3 conv + towers (small levels first) ----
            with (
                tc.tile_pool(name="wk", bufs=1) as wk,
                tc.tile_pool(name="st", bufs=1) as st,
            ):
                def towers_level(i):
                    H, W, _, lvl_off = LEVELS[i]
                    nblk = H * W // 128
                    R2 = 128 // W  # rows per 128-px transpose block
                    fpad = wk.tile([128, MO, 66, 66], BF16, tag="f",
                                   name=f"f{i}pad")
                    zero_border(fpad, H, W)
                    conv3x3(lpad[i], wt[f"fw{i}"], bt[f"fb{i}"], fpad, H, W,
                            relu=False, name=f"fpn{i}", evac_dve=True)

                    regT = st.tile([128, nblk, 12], F32, tag="regT", bufs=2,
                                   name=f"regT{i}")

                    for tower, (w0, b0, w1, b1) in (
                        ("reg", ("rw0", "rb0", "rw1", "rb1")),
                        ("cls", ("cw0", "cb0", "cw1", "cb1")),
                    ):
                        x1 = wk.tile([128, MO, 66, 66], BF16, tag="x1",
                                     name=f"x1_{tower}{i}")
                        zero_border(x1, H, W)
                        conv3x3(fpad, wt[w0], bt[b0], x1, H, W, relu=True,
                                name=f"{tower}0_{i}",
                                evac_dve=(tower == "reg"))
                        x2 = wk.tile([128, MO, 66, 66], BF16, tag="x2",
                                     name=f"x2_{tower}{i}")
                        zero_border(x2, H, W)
                        conv3x3(x1, wt[w1], bt[b1], x2, H, W, relu=True,
                                name=f"{tower}1_{i}",
                                evac_dve=(tower == "reg"))

                        if tower == "reg":
                            # final reg conv, tap-packed: P[tap*12+ch, Y, X]
                            # = w_tap . x2_pad(Y, X) over full contiguous
                            # padded rows (2 matmuls per group), then 9
                            # shifted DVE adds reduce the taps.
                            W2, H2 = W + 2, H + 2
                            ra_max = 512 // W2
                            hh = min(H, 32)  # half-level passes cap SBUF
                            for h0 in range(0, H, hh):
                                h1 = min(h0 + hh, H)
                                reg_lin = st.tile([12, hh, W], F32,
                                                  tag="linr", bufs=1,
                                                  name=f"reglin{i}")
                                a0 = h0
                                while a0 < h1:
                                    ra = min(ra_max, H2 - a0)
                                    ys = min(ra - 2, h1 - a0)
                                    n = ra * W2
                                    accs = []
                                    for j in range(3):
                                        mj = 12 if j == 2 else 128
                                        acc = psr.tile([128, 512], F32,
                                                       tag="psr",
                                                       name=f"regf{i}_ps{j}")
                                        for ki in range(MO):
                                            nc.tensor.matmul(
                                                acc[:mj, :n],
                                                wt["rw2"][:, ki, j, :mj],
                                                x2[:, ki, a0 : a0 + ra,
                                                   0 : W2],
                                                start=(ki == 0),
                                                stop=(ki == MO - 1),
                                            )
                                        accs.append(acc)
                                    dst = reg_lin[:, a0 - h0 : a0 - h0 + ys,
                                                  :]
                                    for t in range(9):
                                        dy, dx = t // 3, t % 3
                                        j, b = t // 4, t % 4
                                        accp = accs[j][:, :n].rearrange(
                                            "p (r w) -> p r w", r=ra
                                        )
                                        src = accp[32 * b : 32 * b + 12,
                                                   dy : dy + ys, dx : dx + W]
                                        if t == 0:
                                            nc.vector.tensor_scalar_add(
                                                dst, src, rb2t[:, 0:1]
                                            )
                                        else:
                                            nc.vector.tensor_add(
                                                dst, dst, src
                                            )
                                    a0 += ys
                                rl_flat = reg_lin.rearrange(
                                    "p h w -> p (h w)"
                                )
                                for blk in range(h0 * W // 128,
                                                 h1 * W // 128):
                                    off = blk * 128 - h0 * W
                                    pt = pst.tile([128, 128], F32,
                                                  tag="pst",
                                                  name=f"regt{i}_ps")
                                    nc.tensor.transpose(
                                        pt[:, :12],
                                        rl_flat[:, off : off + 128],
                                        ident[:12, :12],
                                    )
                                    nc.vector.tensor_copy(
                                        regT[:, blk, :], pt[:, :12]
                                    )
                        else:
                            # final cls conv: 240 (padded 256) outs,
                            # transpose into per-block stage + DMA out
                            for gi, (r0, R) in enumerate(_row_groups(H, W)):
                                n = R * W
                                lins = []
                                for mo in range(MO):
                                    acc = ps.tile([128, 512], F32, tag="ps",
                                                  name=f"clsf{i}_ps")
                                    j = 0
                                    for ki in range(MO):
                                        for t in range(9):
                                            dy, dx = t // 3, t % 3
                                            nc.tensor.matmul(
                                                acc[:, :n],
                                                wt["cw2"][:, ki, t, mo, :],
                                                x2[:, ki,
                                                   r0 + dy : r0 + dy + R,
                                                   dx : dx + W],
                                                start=(j == 0),
                                                stop=(j == MO * 9 - 1),
                                            )
                                            j += 1
                                    lin = st.tile([128, 512], F32, tag="linc",
                                                  bufs=3, name=f"linc{i}")
                                    nc.scalar.activation(
                                        lin[:, :n], acc[:, :n],
                                        mybir.ActivationFunctionType.Identity,
                                        bias=bt["cb2"][:, mo : mo + 1],
                                    )
                                    lins.append(lin)
                                for sub in range(n // 128):
                                    blk = (r0 * W) // 128 + sub
                                    stg = st.tile([128, OUT_CH], F32,
                                                  tag="stg", bufs=3,
                                                  name=f"stg{i}_{blk}")
                                    for mo in range(MO):
                                        used = 128 if mo == 0 else 112
                                        pt = pst.tile([128, 128], F32,
                                                      tag="pst",
                                                      name=f"clst{i}_ps")
                                        nc.tensor.transpose(
                                            pt[:, :used],
                                            lins[mo][:used,
                                                     sub * 128 :
                                                     sub * 128 + 128],
                                            ident[:used, :used],
                                        )
                                        nc.vector.tensor_copy(
                                            stg[:, mo * 128 : mo * 128 + used],
                                            pt[:, :used],
                                        )
                                    nc.vector.tensor_copy(
                                        stg[:, 240:252], regT[:, blk, :]
                                    )
                                    g0 = lvl_off + blk * 128
                                    nc.sync.dma_start(
                                        out[g0 : g0 + 128, :], stg[:]
                                    )

                # Program order drives the scheduler: L2/L1 towers fill the
                # PE while feat3 (4MB) and the level-0 lateral wait on DMA.
                with tc.tile_pool(name="fp3", bufs=1) as fp3:
                    load_weight("fw2")
                    load_weight("rw0")
                    load_weight("rw1")
                    w2 = wp.tile([128, MO, 3, 128], BF16, name="rw2t")
                    nc.sync.dma_start(w2[:], dram["rw2"][:])
                    wt["rw2"] = w2
                    for name in ("cw0", "cw1", "cw2", "fw1"):
                        load_weight(name)
                    load_level(0, fp3)
                    load_weight("fw0")
                    towers_level(2)
                    towers_level(1)
                    lateral_level(0)
                towers_level(0)
    nc.finalize()
    return nc


_NC_CACHE = None


def _get_nc():
    global _NC_CACHE
    if _NC_CACHE is None:
        _NC_CACHE = _build_nc()
    return _NC_CACHE


def _pack3x3(w, o_pad=256):
    """w [O, I, 3, 3] -> [128, I//128, 9, o_pad//128, 128] bf16 (lhsT)."""
    O, I = w.shape[0], w.shape[1]
    arr = np.zeros((128, I // 128, 9, o_pad // 128, 128),
                   dtype=ml_dtypes.bfloat16)
    wb = np.asarray(w, dtype=np.float32).astype(ml_dtypes.bfloat16)
    for ki in range(I // 128):
        blk = wb[:, ki * 128 : (ki + 1) * 128]  # [O, 128, 3, 3]
        for t in range(9):
            bt = blk[:, :, t // 3, t % 3]  # [O, 128]
            for mo in range(o_pad // 128):
                o0, o1 = mo * 128, min((mo + 1) * 128, O)
                if o0 < O:
                    arr[:, ki, t, mo, : o1 - o0] = bt[o0:o1].T
    return arr


def _pack3x3_narrow(w):
    """w [12, 256, 3, 3] -> [128, 2, 3, 128] bf16: pack j holds taps
    t = 4j+b at 32-aligned partition base 32b (M = 32b + out_ch)."""
    arr = np.zeros((128, 2, 3, 128), dtype=ml_dtypes.bfloat16)
    wb = np.asarray(w, dtype=np.float32).astype(ml_dtypes.bfloat16)
    for ki in range(2):
        blk = wb[:, ki * 128 : (ki + 1) * 128]
        for t in range(9):
            j, b = t // 4, t % 4
            arr[:, ki, j, 32 * b : 32 * b + 12] = blk[:, :, t // 3, t % 3].T
    return arr


def _pack1x1(w):
    """w [256, CIN] -> [128, CIN//128, 2, 128] bf16."""
    CIN = w.shape[1]
    arr = np.zeros((128, CIN // 128, 2, 128), dtype=ml_dtypes.bfloat16)
    wb = np.asarray(w, dtype=np.float32).astype(ml_dtypes.bfloat16)
    for ki in range(CIN // 128):
        blk = wb[:, ki * 128 : (ki + 1) * 128]  # [256, 128]
        for mo in range(2):
            arr[:, ki, mo, :] = blk[mo * 128 : (mo + 1) * 128].T
    return arr


def _pack_bias(b, o_pad=256):
    arr = np.zeros((128, o_pad // 128), dtype=np.float32)
    b = np.asarray(b, dtype=np.float32)
    for mo in range(o_pad // 128):
        o0, o1 = mo * 128, min((mo + 1) * 128, b.shape[0])
        if o0 < b.shape[0]:
            arr[: o1 - o0, mo] = b[o0:o1]
    return arr


def _pack_feat(f):
    """f [CIN, H, W] -> [128, CIN//128, H*W] bf16."""
    CIN = f.shape[0]
    fb = np.asarray(f, dtype=np.float32).astype(ml_dtypes.bfloat16)
    return np.ascontiguousarray(
        fb.reshape(CIN // 128, 128, -1).transpose(1, 0, 2)
    )


def _gen_anchors():
    sizes = [32.0, 64.0, 128.0]
    ars = [0.5, 1.0, 2.0]
    hw = [(64, 64), (32, 32), (16, 16)]
    out = []
    for i, ((h, w), bs) in enumerate(zip(hw, sizes)):
        stride = 2.0 ** (i + 3)
        base = np.array(
            [
                [-bs * np.sqrt(a) / 2, -bs / np.sqrt(a) / 2,
                 bs * np.sqrt(a) / 2, bs / np.sqrt(a) / 2]
                for a in ars
            ],
            dtype=np.float32,
        )
        sx = np.arange(w, dtype=np.float32) * stride
        sy = np.arange(h, dtype=np.float32) * stride
        yy, xx = np.meshgrid(sy, sx, indexing="ij")
        shifts = np.stack([xx.ravel(), yy.ravel(), xx.ravel(), yy.ravel()],
                          axis=1)
        anc = (base[None, :, :] + shifts[:, None, :]).reshape(-1, 4)
        out.append(anc)
    return np.concatenate(out, axis=0).astype(np.float32)


def kernel(**inputs):
    nc = _get_nc()

    shared = {
        "lw0": _pack1x1(np.asarray(inputs["lw0"])[:, :, 0, 0]),
        "lw1": _pack1x1(np.asarray(inputs["lw1"])[:, :, 0, 0]),
        "lw2": _pack1x1(np.asarray(inputs["lw2"])[:, :, 0, 0]),
        "fw0": _pack3x3(inputs["fw0"]),
        "fw1": _pack3x3(inputs["fw1"]),
        "fw2": _pack3x3(inputs["fw2"]),
        "cw0": _pack3x3(inputs["cw0"]),
        "cw1": _pack3x3(inputs["cw1"]),
        "cw2": _pack3x3(inputs["cw2"]),  # 240 -> padded 256
        "rw0": _pack3x3(inputs["rw0"]),
        "rw1": _pack3x3(inputs["rw1"]),
        "rw2": _pack3x3_narrow(inputs["rw2"]),
    }
    biases = np.zeros((128, 2 * len(BIAS_NAMES) + 1), dtype=np.float32)
    for idx, name in enumerate(BIAS_NAMES):
        biases[:, 2 * idx : 2 * idx + 2] = _pack_bias(inputs[name])
    biases[0:12, 2 * len(BIAS_NAMES)] = np.asarray(
        inputs["rb2"], dtype=np.float32
    )
    shared["biases"] = biases

    feat_names = ("feat3", "feat4", "feat5")
    in_maps = []
    for b in range(N_CORES):
        m = dict(shared)
        for i, fn in enumerate(feat_names):
            m[f"feat{i}"] = _pack_feat(np.asarray(inputs[fn])[b])
        in_maps.append(m)

    res = run_bass_kernel_spmd(nc, in_maps, list(range(N_CORES)))
    out = np.stack([res.results[b]["out"] for b in range(N_CORES)], axis=0)
    return out, _gen_anchors()
